# revision 1
# baseline (speedup 1.0000x reference)
"""MixerDiffAttention Trainium2 kernel (8-core tensor-parallel over head pairs).

Reference math (B=2, T=2048, D=2048, 16 heads x 256, diff-attention):
  q = x @ Wq.T; k = x @ Wk.T; v = x @ Wv.T   (v as 8 heads x 512)
  q,k: per-head rms_norm then rotary(dim=256)
  split heads into two streams of 8; y = attn1(q1,k1,v) - lam*attn2(q2,k2,v)

Sharding: head-pair i (heads i and i+8 of q/k, v-head i) -> core i.
x replicated; host pre-transposes x and the W shards, pre-rounds to fp32r
(e8m11) so all matmuls run at full PE rate with exact (fp32-accumulate)
arithmetic on the rounded values.

Per-core pipeline (token-major phase -> d-major attention):
  proj (PE fp32r) -> rms stats (DVE ttr) -> normalize (ACT scale, q gets /16)
  -> rotary (DVE, fp16 cos/sin broadcast) -> PE transpose -> kT/qT (f32r)
  scores ST[kt,qt] = kT.T @ qT (PE) -> causal -1e30 add (DVE, diag tiles)
  -> exp (ACT -> f32r p) -> out += p.T @ v, l += p.T @ ones (PE)
  -> y = o1/l1 - lam*o2/l2 (ACT per-partition scale + DVE add) -> DMA out.
"""
import math
from contextlib import ExitStack

import numpy as np

import concourse.bass as bass
import concourse.bacc as bacc
import concourse.tile as tile
import concourse.mybir as mybir
from concourse.bass_utils import run_bass_kernel_spmd

F32 = mybir.dt.float32
F32R = mybir.dt.float32r
F16 = mybir.dt.float16
AF = mybir.ActivationFunctionType
ALU = mybir.AluOpType

B = 2
D = 2048
N_HEADS = 16
HEAD_DIM = 256
OF = 512                      # per-core q/k/v feature width (2 heads x 256)
KC = D // 128                 # contraction chunks for projections
LAMBDA_INIT = 0.8 - 0.6 * math.exp(-0.3 * 0)
EPS = float(np.finfo(np.float32).eps)
NEG = -1.0e30


def round_fp32r(x: np.ndarray) -> np.ndarray:
    """RNE-round fp32 array to fp32r (e8m11, low 12 bits zero)."""
    u = np.ascontiguousarray(x, dtype=np.float32).view(np.uint32)
    low = u & np.uint32(0xFFF)
    base = u & ~np.uint32(0xFFF)
    round_up = (low > 0x800) | ((low == 0x800) & (((u >> 12) & 1) == 1))
    return (base + (round_up.astype(np.uint32) << 12)).view(np.float32)


_TABLES_PATCHED = False


def _patch_act_tables():
    """Force every activation into natural_log_exp_and_others so the kernel
    needs exactly one ACT table load (Sqrt is replaced by exp(-0.5*ln))."""
    global _TABLES_PATCHED
    if _TABLES_PATCHED:
        return
    import concourse.hw_specs as hw_specs
    mine = {AF.Exp, AF.Ln, AF.Square, AF.Copy, AF.Identity}
    orig = hw_specs.get_activation_tables

    def patched(arch):
        out = {}
        for name, funcs in orig(arch).items():
            out[name] = funcs if name == "natural_log_exp_and_others" \
                else (funcs - mine)
        return out

    bacc.get_activation_tables = patched
    _TABLES_PATCHED = True


def build_nc(T: int = 2048):
    """Build the single-core SPMD program for per-batch token count T."""
    NT = T // 128            # token tiles per batch
    NQC = T // 256           # query chunks per batch
    _patch_act_tables()
    nc = bacc.Bacc("TRN2", target_bir_lowering=False, debug=False)

    xT = nc.dram_tensor("xT", [D, B * T], F32R, kind="ExternalInput").ap()
    wqT = nc.dram_tensor("wqT", [D, OF], F32R, kind="ExternalInput").ap()
    wkT = nc.dram_tensor("wkT", [D, OF], F32R, kind="ExternalInput").ap()
    wvT = nc.dram_tensor("wvT", [D, OF], F32R, kind="ExternalInput").ap()
    cosd = nc.dram_tensor("cosd", [128, NT * 128], F16, kind="ExternalInput").ap()
    sind = nc.dram_tensor("sind", [128, NT * 128], F16, kind="ExternalInput").ap()
    identd = nc.dram_tensor("identd", [128, 128], F32R, kind="ExternalInput").ap()
    onesd = nc.dram_tensor("onesd", [128, 1], F32, kind="ExternalInput").ap()
    onesrd = nc.dram_tensor("onesrd", [128, 1], F32R, kind="ExternalInput").ap()
    maskd = nc.dram_tensor("maskd", [128, 384], F32, kind="ExternalInput").ap()
    lamd = nc.dram_tensor("lamd", [128, 1], F32, kind="ExternalInput").ap()
    out = nc.dram_tensor("out", [B, T, OF], F32, kind="ExternalOutput").ap()

    xT_r = xT.rearrange("(c p) t -> p c t", p=128)      # [128, KC, B*T]
    wq_r = wqT.rearrange("(c p) n -> p c n", p=128)     # [128, KC, OF]
    wk_r = wkT.rearrange("(c p) n -> p c n", p=128)
    wv_r = wvT.rearrange("(c p) n -> p c n", p=128)

    with tile.TileContext(nc) as tc, ExitStack() as ctx:
        # --- constant pools (loaded once) ---
        cpool = ctx.enter_context(tc.tile_pool(name="consts", bufs=1))
        cos_t = cpool.tile([128, NT, 128], F16, tag="cos")
        sin_t = cpool.tile([128, NT, 128], F16, tag="sin")
        id_t = cpool.tile([128, 128], F32R, tag="ident")
        ones_t = cpool.tile([128, 1], F32, tag="ones")
        onesr_t = cpool.tile([128, 1], F32R, tag="onesr")
        mask_t = cpool.tile([128, 384], F32, tag="mask")
        lam_t = cpool.tile([128, 1], F32, tag="lam")
        eps_t = cpool.tile([128, 1], F32, tag="eps")
        ln16_t = cpool.tile([128, 1], F32, tag="ln16")
        nc.vector.memset(eps_t[:], EPS)
        nc.vector.memset(ln16_t[:], -math.log(16.0))

        def load_consts():
            nc.sync.dma_start(cos_t[:],
                              cosd.rearrange("p (n f) -> p n f", f=128))
            nc.sync.dma_start(sin_t[:],
                              sind.rearrange("p (n f) -> p n f", f=128))
            nc.sync.dma_start(id_t[:], identd)
            nc.sync.dma_start(ones_t[:], onesd)
            nc.sync.dma_start(onesr_t[:], onesrd)
            nc.sync.dma_start(mask_t[:], maskd)
            nc.sync.dma_start(lam_t[:], lamd)

        # --- weight pools: slot w1 = wk (A) / wq (B); slot w2 = wv (A) ---
        wpool = ctx.enter_context(tc.tile_pool(name="weights", bufs=1))
        # --- per-batch key/value caches ---
        kvpool = ctx.enter_context(tc.tile_pool(name="kv", bufs=1))
        # --- streaming pools ---
        xpool = ctx.enter_context(tc.tile_pool(name="x", bufs=2))
        tpool = ctx.enter_context(tc.tile_pool(name="t", bufs=3))
        qtpool = ctx.enter_context(tc.tile_pool(name="qt", bufs=2))
        ppool = ctx.enter_context(tc.tile_pool(name="p", bufs=3))
        spool = ctx.enter_context(tc.tile_pool(name="stats", bufs=4))
        ypool = ctx.enter_context(tc.tile_pool(name="y", bufs=2))
        yspool = ctx.enter_context(tc.tile_pool(name="ysave", bufs=2))
        # --- PSUM pools: big 2 + out 2 + l 1 + (st|tr shared) 3 = 8 banks ---
        bigps = ctx.enter_context(tc.tile_pool(name="bigps", bufs=2, space="PSUM"))
        outps = ctx.enter_context(tc.tile_pool(name="outps", bufs=2, space="PSUM"))
        lps = ctx.enter_context(tc.tile_pool(name="lps", bufs=1, space="PSUM"))
        strps = ctx.enter_context(tc.tile_pool(name="strps", bufs=3, space="PSUM"))

        def load_x(b, t0):
            """xT chunk tiles, split in two for earlier matmul start."""
            xa = xpool.tile([128, KC // 2, 128], F32R, tag="xta")
            xb = xpool.tile([128, KC // 2, 128], F32R, tag="xtb")
            t0 += b * T
            nc.sync.dma_start(xa[:], xT_r[:, 0:KC // 2, t0:t0 + 128])
            nc.sync.dma_start(xb[:], xT_r[:, KC // 2:KC, t0:t0 + 128])
            return (xa, xb)

        def proj(xt, wg):
            """PSUM [tok 128, OF] = x-tile @ W.T via KC accumulating matmuls.

            wg: list of 4 weight tiles [128, KC//4, OF] (chunk groups).
            """
            ps = bigps.tile([128, OF], F32, tag="big", name="projps")
            half = KC // 2
            for c in range(KC):
                nc.tensor.matmul(ps[:], xt[c // half][:, c % half],
                                 wg[c // 4][:, c % 4],
                                 start=(c == 0), stop=(c == KC - 1))
            return ps

        def qk_process(ps, tt, is_q, dst, dst_col):
            """rms-normalize + rotary + transpose [tok,512] psum -> dst f32r.

            dst: SBUF tile [128, 4, X]; writes cols [dst_col, dst_col+128).
            Stats and rotary multiplies read PSUM directly (independent early
            ops) so the proj bank frees fast; normalize happens post-rotary.
            """
            # sum of squares per head: ACT Square + free-dim accumulate
            tsq = tpool.tile([128, OF], F32, tag="tsq")
            ss = spool.tile([128, 2], F32, tag="ss")
            for h in range(2):
                nc.scalar.activation(
                    tsq[:, h * 256:(h + 1) * 256],
                    ps[:, h * 256:(h + 1) * 256],
                    AF.Square, accum_out=ss[:, h:h + 1])
            # rotary multiplies straight from PSUM
            cos_b = cos_t[:, tt:tt + 1, :].broadcast_to([128, 4, 128])
            sin_b = sin_t[:, tt:tt + 1, :].broadcast_to([128, 4, 128])
            t1t = tpool.tile([128, OF], F32, tag="t1")
            t2t = tpool.tile([128, OF], F32, tag="t2")
            ps4 = ps.rearrange("p (a f) -> p a f", f=128)
            t1 = t1t.rearrange("p (a f) -> p a f", f=128)
            t2 = t2t.rearrange("p (a f) -> p a f", f=128)
            nc.vector.tensor_tensor(t1[:], ps4[:], cos_b, ALU.mult)
            nc.vector.tensor_tensor(t2[:], ps4[:], sin_b, ALU.mult)
            # r = rsqrt(mean + eps) = exp(-0.5*ln(mean+eps)); q also /16
            lg = spool.tile([128, 2], F32, tag="lg")
            nc.scalar.activation(lg[:], ss[:], AF.Ln, scale=1.0 / 256.0,
                                 bias=eps_t[:, 0:1])
            r = spool.tile([128, 2], F32, tag="r")
            nc.scalar.activation(r[:], lg[:], AF.Exp, scale=-0.5,
                                 bias=(ln16_t[:, 0:1] if is_q else 0.0))
            # rotary combine (into tsq, dead after squares) then normalize
            yt = tsq
            yv = yt.rearrange("p (h u f) -> p h u f", h=2, f=128)
            t1v = t1t.rearrange("p (h u f) -> p h u f", h=2, f=128)
            t2v = t2t.rearrange("p (h u f) -> p h u f", h=2, f=128)
            nc.vector.tensor_tensor(yv[:, :, 0], t1v[:, :, 0], t2v[:, :, 1], ALU.add)
            nc.vector.tensor_tensor(yv[:, :, 1], t1v[:, :, 1], t2v[:, :, 0],
                                    ALU.subtract)
            ynt = tpool.tile([128, OF], F32R, tag="yn")
            yn = ynt[:]
            for h in range(2):
                nc.scalar.mul(yn[:, h * 256:(h + 1) * 256],
                              yt[:, h * 256:(h + 1) * 256], r[:, h:h + 1])
            # transpose 4 d-chunks through one PSUM bank, copy out as f32r
            tp = strps.tile([128, 4, 128], F32R, tag="str", name="trtile")
            for c in range(4):
                nc.tensor.transpose(tp[:, c], yn[:, c * 128:(c + 1) * 128],
                                    id_t[:])
            nc.vector.tensor_copy(dst[:, :, dst_col:dst_col + 128],
                                  tp[:].bitcast(F32))

        for b in range(B):
            w1 = [wpool.tile([128, 4, OF], F32R, tag=f"w1g{g}",
                             name=f"w1g{g}") for g in range(4)]
            w2 = [wpool.tile([128, 4, OF], F32R, tag=f"w2g{g}",
                             name=f"w2g{g}") for g in range(4)]
            for g in range(4):
                cs = slice(g * 4, (g + 1) * 4)
                nc.sync.dma_start(w1[g][:], wk_r[:, cs])
                nc.sync.dma_start(w2[g][:], wv_r[:, cs])
            kT = kvpool.tile([128, 4, T], F32R, tag="kT")
            vsb = kvpool.tile([128, NT, OF], F32R, tag="v")

            # --- Phase A: keys and values fused (one x load per tile) ---
            for tt in range(NT):
                xt = load_x(b, tt * 128)
                if b == 0 and tt == 0:
                    load_consts()   # queued behind the critical w/x loads
                kps = proj(xt, w1)
                vps = proj(xt, w2)
                nc.vector.tensor_copy(vsb[:, tt].bitcast(F32R), vps[:])
                qk_process(kps, tt, False, kT, tt * 128)

            # wq reuses wk's slots; split DMA so it lands fast after phase A
            wq_t = [wpool.tile([128, 4, OF], F32R, tag=f"w1g{g}",
                               name=f"wqg{g}") for g in range(4)]
            for g in range(4):
                cs = slice(g * 4, (g + 1) * 4)
                nc.sync.dma_start(wq_t[g][:], wq_r[:, cs])

            # --- Phase B: queries + attention (q-proj pipelined one qc ahead)
            def q_proj_chunk(qc):
                qT = qtpool.tile([128, 4, 256], F32R, tag="qT", name=f"qT{qc}")
                for u in range(2):
                    tt = qc * 2 + u
                    xt = load_x(b, tt * 128)
                    qps = proj(xt, wq_t)
                    qk_process(qps, tt, True, qT, u * 128)
                return qT

            qT_next = q_proj_chunk(0)
            for qc in range(NQC):
                qT = qT_next
                if qc + 1 < NQC:
                    qT_next = q_proj_chunk(qc + 1)

                nkt = 2 * (qc + 1)
                ysv = [None, None]
                for s in range(2):
                    op = [outps.tile([128, OF], F32, tag="o", name=f"op{u_}")
                          for u_ in range(2)]
                    lp = lps.tile([128, 2, NT], F32, tag="l")
                    for kt in range(nkt):
                        # kt == nkt-1 is fully masked for u=0: only the u=1
                        # half of scores/exp/pv/l is needed there.
                        last = (kt == nkt - 1)
                        cols = slice(128, 256) if last else slice(0, 256)
                        st = strps.tile([128, 256], F32, tag="str",
                                        name="sttile")
                        for c2 in range(2):
                            nc.tensor.matmul(
                                st[:],
                                kT[:, 2 * s + c2, kt * 128:(kt + 1) * 128],
                                qT[:, 2 * s + c2, :],
                                start=(c2 == 0), stop=(c2 == 1))
                        o = (kt - 2 * qc) * 128
                        if o >= 0:
                            # triangular part covers one u-half only
                            mc = slice(0, 128) if o == 0 else slice(128, 256)
                            nc.vector.tensor_tensor(
                                st[:, mc], st[:, mc], mask_t[:, 128:256],
                                ALU.add)
                        p = ppool.tile([128, 256], F32R, tag="p")
                        nc.scalar.activation(p[:, cols], st[:, cols], AF.Exp)
                        for u in range(2):
                            if last and u == 0:
                                continue
                            lastu = nkt - 1 if u == 1 else nkt - 2
                            nc.tensor.matmul(
                                op[u][:], p[:, u * 128:(u + 1) * 128], vsb[:, kt],
                                start=(kt == 0), stop=(kt == lastu))
                            nc.tensor.matmul(
                                lp[:, u, kt:kt + 1],
                                p[:, u * 128:(u + 1) * 128].bitcast(F32),
                                ones_t[:],
                                start=True, stop=True)
                    lsum = spool.tile([128, 2], F32, tag="lsum")
                    nc.vector.reduce_sum(lsum[:, 0:1],
                                         lp[:, 0:1, 0:max(nkt - 1, 1)],
                                         axis=mybir.AxisListType.X)
                    nc.vector.reduce_sum(lsum[:, 1:2], lp[:, 1:2, 0:nkt],
                                         axis=mybir.AxisListType.X)
                    linv = spool.tile([128, 2], F32, tag="linv")
                    nc.vector.reciprocal(linv[:], lsum[:])
                    if s == 1:
                        nlam = spool.tile([128, 2], F32, tag="nlam")
                        nc.vector.tensor_scalar_mul(
                            nlam[:], linv[:], lam_t[:, 0:1])
                        linv = nlam
                    for u in range(2):
                        if s == 0:
                            ysv[u] = yspool.tile([128, OF], F32, tag="ys",
                                                 name=f"ys{u}")
                            nc.scalar.mul(ysv[u][:], op[u][:],
                                          linv[:, u:u + 1])
                        else:
                            yb = ypool.tile([128, OF], F32, tag="yb")
                            nc.scalar.mul(yb[:], op[u][:], linv[:, u:u + 1])
                            yf = ypool.tile([128, OF], F32, tag="yf")
                            nc.vector.tensor_tensor(yf[:], ysv[u][:], yb[:],
                                                    ALU.subtract)
                            t0 = qc * 256 + u * 128
                            nc.sync.dma_start(out[b, t0:t0 + 128, :], yf[:])
    nc.compile()
    return nc


def make_in_maps(x, Wq, Wk, Wv, lam, T):
    """Host-side sharding + layout prep. Returns list of 8 in_maps."""
    NT = T // 128
    xf = np.ascontiguousarray(x.reshape(B * T, D).T)        # [D, B*T]
    xf = round_fp32r(xf)
    t = np.arange(T, dtype=np.float64)
    inv = 1.0 / (10000.0 ** (np.arange(0, HEAD_DIM, 2, dtype=np.float64)
                             / HEAD_DIM))
    fr = np.outer(t, inv)                                    # [T, 128]
    cos = np.cos(fr).astype(np.float32)
    sin = np.sin(fr).astype(np.float32)
    # [128, NT*128]: row p, col tt*128+f  ->  cos[tt*128+p, f]
    cos_sb = np.ascontiguousarray(
        cos.reshape(NT, 128, 128).transpose(1, 0, 2).reshape(128, NT * 128)
    ).astype(np.float16)
    sin_sb = np.ascontiguousarray(
        sin.reshape(NT, 128, 128).transpose(1, 0, 2).reshape(128, NT * 128)
    ).astype(np.float16)
    ident = np.eye(128, dtype=np.float32)
    ones1 = np.ones((128, 1), np.float32)
    ii = np.arange(128).reshape(128, 1)
    mm_ = np.arange(384).reshape(1, 384) - 128
    maskneg = np.where(mm_ >= ii, 0.0, NEG).astype(np.float32)
    lam_np = np.full((128, 1), lam, np.float32)

    common = {"xT": xf, "cosd": cos_sb, "sind": sin_sb, "identd": ident,
              "onesd": ones1, "onesrd": ones1, "maskd": maskneg,
              "lamd": lam_np}
    in_maps = []
    for i in range(8):
        wq_sh = np.concatenate(
            [Wq[i * 256:(i + 1) * 256], Wq[(i + 8) * 256:(i + 9) * 256]], 0)
        wk_sh = np.concatenate(
            [Wk[i * 256:(i + 1) * 256], Wk[(i + 8) * 256:(i + 9) * 256]], 0)
        wv_sh = Wv[i * 512:(i + 1) * 512]
        m = dict(common)
        m["wqT"] = round_fp32r(np.ascontiguousarray(wq_sh.T))
        m["wkT"] = round_fp32r(np.ascontiguousarray(wk_sh.T))
        m["wvT"] = round_fp32r(np.ascontiguousarray(wv_sh.T))
        in_maps.append(m)
    return in_maps


_NC_CACHE: dict = {}


def run_cores(x, Wq, Wk, Wv, lambda_q1, lambda_k1, lambda_q2, lambda_k2,
              T=2048, **spmd_kwargs):
    lam1 = np.exp(np.float32(np.dot(lambda_q1.astype(np.float32),
                                    lambda_k1.astype(np.float32))))
    lam2 = np.exp(np.float32(np.dot(lambda_q2.astype(np.float32),
                                    lambda_k2.astype(np.float32))))
    lam = np.float32(lam1 - lam2 + np.float32(LAMBDA_INIT))
    if T not in _NC_CACHE:
        _NC_CACHE[T] = build_nc(T)
    nc = _NC_CACHE[T]
    in_maps = make_in_maps(np.asarray(x), np.asarray(Wq), np.asarray(Wk),
                           np.asarray(Wv), lam, T)
    res = run_bass_kernel_spmd(nc, in_maps, core_ids=list(range(8)),
                               **spmd_kwargs)
    shards = [res.results[i]["out"] for i in range(8)]       # [B,T,512] each
    y = np.stack(shards, axis=2).reshape(B, T, N_HEADS * HEAD_DIM)
    return y, res


def kernel(x, Wq, Wk, Wv, lambda_q1, lambda_k1, lambda_q2, lambda_k2):
    y, _ = run_cores(x, Wq, Wk, Wv, lambda_q1, lambda_k1, lambda_q2,
                     lambda_k2, T=x.shape[1])
    return y.astype(np.float32)



# revision 5
# speedup vs baseline: 1.1406x; 1.1406x over previous
"""MixerDiffAttention Trainium2 kernel (8-core tensor-parallel over head pairs).

Reference math (B=2, T=2048, D=2048, 16 heads x 256, diff-attention):
  q = x @ Wq.T; k = x @ Wk.T; v = x @ Wv.T   (v as 8 heads x 512)
  q,k: per-head rms_norm then rotary(dim=256)
  split heads into two streams of 8; y = attn1(q1,k1,v) - lam*attn2(q2,k2,v)

Sharding: head-pair i (heads i and i+8 of q/k, v-head i) -> core i.
x replicated; host pre-quantizes x and the W shards to fp8e4 plus fp8e4
residuals, so projections run as DoubleRow (0.5 cyc/row, 256-contraction)
matmul chains:
  q,k: x8@W8 + xr@W8 + x8@Wr   (3 chains; ~0.2% effective error)
  v:   x8@W8 + xr@W8           (2 chains; W-quant noise ok for v)
W is pre-scaled by 64 on host; q/k absorb it in rms_norm, v divides at the
PSUM->SBUF copy. Attention operands (kT, qT, p, v) are fp16: full-rate PE
matmuls with ~0.05% quantization noise. p = exp(score - 5.5) keeps fp16 in
range; the e^-5.5 cancels in the final p@v / l division.

Per-core pipeline (token-major phase -> d-major attention):
  proj (PE fp8-DR) -> rms stats (ACT Square+accum) -> rotary (DVE, f16
  cos/sin broadcast) -> normalize (ACT scale, q gets /16) -> PE transpose
  (f16) -> kT/qT f16
  scores ST[kt,qt] = kT.T @ qT (PE f16) -> causal -1e30 add (DVE, diag tiles)
  -> exp-5.5 (ACT -> f16 p) -> out += p.T @ v, l += p.T @ ones (PE)
  -> y = o1/l1 - lam*o2/l2 (ACT per-partition scale + DVE sub) -> DMA out.
"""
import math
from contextlib import ExitStack

import numpy as np
import ml_dtypes

import concourse.bass as bass
import concourse.bacc as bacc
import concourse.tile as tile
import concourse.mybir as mybir
from concourse.bass_utils import run_bass_kernel_spmd

F32 = mybir.dt.float32
F16 = mybir.dt.float16
FP8 = mybir.dt.float8e4
AF = mybir.ActivationFunctionType
ALU = mybir.AluOpType

NP_FP8 = ml_dtypes.float8_e4m3fn

B = 2
D = 2048
N_HEADS = 16
HEAD_DIM = 256
OF = 512                      # per-core q/k/v feature width (2 heads x 256)
NG = D // 256                 # DoubleRow contraction-pair groups (8)
WS = 64.0                     # host weight prescale
LAMBDA_INIT = 0.8 - 0.6 * math.exp(-0.3 * 0)
EPS = float(np.finfo(np.float32).eps)
NEG = -1.0e30
PBIAS = -5.5                  # exp bias so p fits fp16 (cancels in o/l)


def q8(x: np.ndarray) -> np.ndarray:
    """Quantize to TRN e4m3 (clip to +-240, RNE)."""
    return np.clip(np.asarray(x, np.float32), -240.0, 240.0).astype(NP_FP8)


_TABLES_PATCHED = False


def _patch_act_tables():
    """Force every activation into natural_log_exp_and_others so the kernel
    needs exactly one ACT table load."""
    global _TABLES_PATCHED
    if _TABLES_PATCHED:
        return
    import concourse.hw_specs as hw_specs
    mine = {AF.Exp, AF.Ln, AF.Square, AF.Copy, AF.Identity}
    orig = hw_specs.get_activation_tables

    def patched(arch):
        out = {}
        for name, funcs in orig(arch).items():
            out[name] = funcs if name == "natural_log_exp_and_others" \
                else (funcs - mine)
        return out

    bacc.get_activation_tables = patched
    _TABLES_PATCHED = True


def build_nc(T: int = 2048):
    """Build the single-core SPMD program for per-batch token count T."""
    NT = T // 128            # token tiles per batch
    NQC = T // 256           # query chunks per batch
    _patch_act_tables()
    nc = bacc.Bacc("TRN2", target_bir_lowering=False, debug=False)

    x8d = nc.dram_tensor("x8d", [D, B * T], FP8, kind="ExternalInput").ap()
    xrd = nc.dram_tensor("xrd", [D, B * T], FP8, kind="ExternalInput").ap()
    wd = {}
    for nm in ("wq8", "wqr", "wk8", "wkr", "wv8", "wvr"):
        wd[nm] = nc.dram_tensor(nm, [D, OF], FP8, kind="ExternalInput").ap()
    cosd = nc.dram_tensor("cosd", [128, NT * 128], F16, kind="ExternalInput").ap()
    sind = nc.dram_tensor("sind", [128, NT * 128], F16, kind="ExternalInput").ap()
    identd = nc.dram_tensor("identd", [128, 128], F16, kind="ExternalInput").ap()
    onesd = nc.dram_tensor("onesd", [128, 1], F16, kind="ExternalInput").ap()
    maskd = nc.dram_tensor("maskd", [128, 384], F32, kind="ExternalInput").ap()
    lamd = nc.dram_tensor("lamd", [128, 1], F32, kind="ExternalInput").ap()
    out = nc.dram_tensor("out", [B, T, OF], F32, kind="ExternalOutput").ap()

    # [128, (g two), t]: contraction chunk c = 2g+two, row p -> d = c*128+p
    x8_r = x8d.rearrange("(g two p) t -> p g two t", p=128, two=2)
    xr_r = xrd.rearrange("(g two p) t -> p g two t", p=128, two=2)
    w_r = {nm: ap.rearrange("(g two p) n -> p g two n", p=128, two=2)
           for nm, ap in wd.items()}

    with tile.TileContext(nc) as tc, ExitStack() as ctx:
        # --- constant pools (loaded once) ---
        cpool = ctx.enter_context(tc.tile_pool(name="consts", bufs=1))
        cos_t = cpool.tile([128, NT, 128], F16, tag="cos")
        sin_t = cpool.tile([128, NT, 128], F16, tag="sin")
        id_t = cpool.tile([128, 128], F16, tag="ident")
        ones_t = cpool.tile([128, 1], F16, tag="ones")
        mask_t = cpool.tile([128, 384], F32, tag="mask")
        lam_t = cpool.tile([128, 1], F32, tag="lam")
        eps_t = cpool.tile([128, 1], F32, tag="eps")
        ln16_t = cpool.tile([128, 1], F32, tag="ln16")
        pbias_t = cpool.tile([128, 1], F32, tag="pbias")
        nc.vector.memset(eps_t[:], EPS)
        nc.vector.memset(ln16_t[:], -math.log(16.0))
        nc.vector.memset(pbias_t[:], PBIAS)

        def load_consts():
            nc.sync.dma_start(cos_t[:],
                              cosd.rearrange("p (n f) -> p n f", f=128))
            nc.sync.dma_start(sin_t[:],
                              sind.rearrange("p (n f) -> p n f", f=128))
            nc.sync.dma_start(id_t[:], identd)
            nc.sync.dma_start(ones_t[:], onesd)
            nc.sync.dma_start(mask_t[:], maskd)
            nc.sync.dma_start(lam_t[:], lamd)

        # --- weights: all six fp8 shards resident for the whole kernel ---
        wpool = ctx.enter_context(tc.tile_pool(name="weights", bufs=1))
        wt = {}
        for nm in ("wq8", "wqr", "wk8", "wkr", "wv8", "wvr"):
            wt[nm] = wpool.tile([128, NG, 2, OF], FP8, tag=nm, name=nm)

        # --- per-batch resident x (fp8 + residual) ---
        xpool = ctx.enter_context(tc.tile_pool(name="x", bufs=1))
        x8sb = xpool.tile([128, NG, 2, T], FP8, tag="x8sb", name="x8sb")
        xrsb = xpool.tile([128, NG, 2, T], FP8, tag="xrsb", name="xrsb")

        # --- per-batch key/value caches ---
        kvpool = ctx.enter_context(tc.tile_pool(name="kv", bufs=1))
        # --- streaming pools ---
        tpool = ctx.enter_context(tc.tile_pool(name="t", bufs=3))
        qtpool = ctx.enter_context(tc.tile_pool(name="qt", bufs=2))
        ppool = ctx.enter_context(tc.tile_pool(name="p", bufs=3))
        spool = ctx.enter_context(tc.tile_pool(name="stats", bufs=4))
        ypool = ctx.enter_context(tc.tile_pool(name="y", bufs=2))
        yspool = ctx.enter_context(tc.tile_pool(name="ysave", bufs=2))
        # --- PSUM pools: big 2 + out 2 + l 1 + (st|tr shared) 3 = 8 banks ---
        bigps = ctx.enter_context(tc.tile_pool(name="bigps", bufs=2, space="PSUM"))
        outps = ctx.enter_context(tc.tile_pool(name="outps", bufs=2, space="PSUM"))
        lps = ctx.enter_context(tc.tile_pool(name="lps", bufs=1, space="PSUM"))
        strps = ctx.enter_context(tc.tile_pool(name="strps", bufs=3, space="PSUM"))

        def proj(t0, chains):
            """PSUM [tok 128, OF] via DoubleRow accumulation chains.

            chains: list of (x_tile, w_tile); each contributes NG DR matmuls.
            """
            ps = bigps.tile([128, OF], F32, tag="big", name="projps")
            n = len(chains) * NG
            i = 0
            for xt, wtile in chains:
                for g in range(NG):
                    nc.tensor.matmul(
                        ps[:], xt[:, g, :, t0:t0 + 128], wtile[:, g],
                        start=(i == 0), stop=(i == n - 1),
                        perf_mode=mybir.MatmulPerfMode.DoubleRow)
                    i += 1
            return ps

        def qk_process(ps, tt, is_q, dst, dst_col):
            """rms-normalize + rotary + transpose [tok,512] psum -> dst f16.

            dst: SBUF tile [128, 4, X]; writes cols [dst_col, dst_col+128).
            """
            # sum of squares per head: ACT Square + free-dim accumulate
            tsq = tpool.tile([128, OF], F32, tag="tsq")
            ss = spool.tile([128, 2], F32, tag="ss")
            for h in range(2):
                nc.scalar.activation(
                    tsq[:, h * 256:(h + 1) * 256],
                    ps[:, h * 256:(h + 1) * 256],
                    AF.Square, accum_out=ss[:, h:h + 1])
            # rotary multiplies straight from PSUM
            cos_b = cos_t[:, tt:tt + 1, :].broadcast_to([128, 4, 128])
            sin_b = sin_t[:, tt:tt + 1, :].broadcast_to([128, 4, 128])
            t1t = tpool.tile([128, OF], F32, tag="t1")
            t2t = tpool.tile([128, OF], F32, tag="t2")
            ps4 = ps.rearrange("p (a f) -> p a f", f=128)
            t1 = t1t.rearrange("p (a f) -> p a f", f=128)
            t2 = t2t.rearrange("p (a f) -> p a f", f=128)
            nc.vector.tensor_tensor(t1[:], ps4[:], cos_b, ALU.mult)
            nc.vector.tensor_tensor(t2[:], ps4[:], sin_b, ALU.mult)
            # r = rsqrt(mean + eps) = exp(-0.5*ln(mean+eps)); q also /16
            lg = spool.tile([128, 2], F32, tag="lg")
            nc.scalar.activation(lg[:], ss[:], AF.Ln, scale=1.0 / 256.0,
                                 bias=eps_t[:, 0:1])
            r = spool.tile([128, 2], F32, tag="r")
            nc.scalar.activation(r[:], lg[:], AF.Exp, scale=-0.5,
                                 bias=(ln16_t[:, 0:1] if is_q else 0.0))
            # rotary combine (into tsq, dead after squares) then normalize
            yt = tsq
            yv = yt.rearrange("p (h u f) -> p h u f", h=2, f=128)
            t1v = t1t.rearrange("p (h u f) -> p h u f", h=2, f=128)
            t2v = t2t.rearrange("p (h u f) -> p h u f", h=2, f=128)
            nc.vector.tensor_tensor(yv[:, :, 0], t1v[:, :, 0], t2v[:, :, 1], ALU.add)
            nc.vector.tensor_tensor(yv[:, :, 1], t1v[:, :, 1], t2v[:, :, 0],
                                    ALU.subtract)
            ynt = tpool.tile([128, OF], F16, tag="yn")
            yn = ynt[:]
            for h in range(2):
                nc.scalar.mul(yn[:, h * 256:(h + 1) * 256],
                              yt[:, h * 256:(h + 1) * 256], r[:, h:h + 1])
            # transpose 4 d-chunks through one PSUM bank, copy out as f16
            tp = strps.tile([128, 4, 128], F16, tag="str", name="trtile")
            for c in range(4):
                nc.tensor.transpose(tp[:, c], yn[:, c * 128:(c + 1) * 128],
                                    id_t[:])
            nc.vector.tensor_copy(dst[:, :, dst_col:dst_col + 128], tp[:])

        for b in range(B):
            # load this batch's x slabs (fp8 + residual)
            ts_ = slice(b * T, (b + 1) * T)
            nc.sync.dma_start(x8sb[:], x8_r[:, :, :, ts_])
            nc.sync.dma_start(xrsb[:], xr_r[:, :, :, ts_])
            if b == 0:
                for nm in ("wk8", "wkr", "wv8", "wvr", "wq8", "wqr"):
                    nc.sync.dma_start(wt[nm][:], w_r[nm])
                load_consts()

            kT = kvpool.tile([128, 4, T], F16, tag="kT")
            vsb = kvpool.tile([128, NT, OF], F16, tag="v")

            # --- Phase A: keys and values ---
            for tt in range(NT):
                t0 = tt * 128
                kps = proj(t0, [(x8sb, wt["wk8"]), (xrsb, wt["wk8"]),
                                (x8sb, wt["wkr"])])
                vps = proj(t0, [(x8sb, wt["wv8"]), (xrsb, wt["wv8"]),
                                (x8sb, wt["wvr"])])
                nc.scalar.mul(vsb[:, tt], vps[:], 1.0 / WS)
                qk_process(kps, tt, False, kT, tt * 128)

            # --- Phase B: queries + attention (q-proj pipelined one qc ahead)
            def q_proj_chunk(qc):
                qT = qtpool.tile([128, 4, 256], F16, tag="qT", name=f"qT{qc}")
                for u in range(2):
                    tt = qc * 2 + u
                    qps = proj(tt * 128, [(x8sb, wt["wq8"]), (xrsb, wt["wq8"]),
                                          (x8sb, wt["wqr"])])
                    qk_process(qps, tt, True, qT, u * 128)
                return qT

            qT_next = q_proj_chunk(0)
            for qc in range(NQC):
                qT = qT_next
                if qc + 1 < NQC:
                    qT_next = q_proj_chunk(qc + 1)

                nkt = 2 * (qc + 1)
                ysv = [None, None]
                for s in range(2):
                    op = [outps.tile([128, OF], F32, tag="o", name=f"op{u_}")
                          for u_ in range(2)]
                    lp = lps.tile([128, 2, NT], F32, tag="l")
                    for kt in range(nkt):
                        # kt == nkt-1 is fully masked for u=0: only the u=1
                        # half of scores/exp/pv/l is needed there.
                        last = (kt == nkt - 1)
                        cols = slice(128, 256) if last else slice(0, 256)
                        st = strps.tile([128, 256], F32, tag="str",
                                        name="sttile")
                        for c2 in range(2):
                            nc.tensor.matmul(
                                st[:],
                                kT[:, 2 * s + c2, kt * 128:(kt + 1) * 128],
                                qT[:, 2 * s + c2, :],
                                start=(c2 == 0), stop=(c2 == 1))
                        o = (kt - 2 * qc) * 128
                        if o >= 0:
                            # triangular part covers one u-half only
                            mc = slice(0, 128) if o == 0 else slice(128, 256)
                            nc.vector.tensor_tensor(
                                st[:, mc], st[:, mc], mask_t[:, 128:256],
                                ALU.add)
                        p = ppool.tile([128, 256], F16, tag="p")
                        nc.scalar.activation(p[:, cols], st[:, cols], AF.Exp,
                                             bias=pbias_t[:, 0:1])
                        for u in range(2):
                            if last and u == 0:
                                continue
                            lastu = nkt - 1 if u == 1 else nkt - 2
                            nc.tensor.matmul(
                                op[u][:], p[:, u * 128:(u + 1) * 128], vsb[:, kt],
                                start=(kt == 0), stop=(kt == lastu))
                            nc.tensor.matmul(
                                lp[:, u, kt:kt + 1],
                                p[:, u * 128:(u + 1) * 128],
                                ones_t[:],
                                start=True, stop=True)
                    lsum = spool.tile([128, 2], F32, tag="lsum")
                    nc.vector.reduce_sum(lsum[:, 0:1],
                                         lp[:, 0:1, 0:max(nkt - 1, 1)],
                                         axis=mybir.AxisListType.X)
                    nc.vector.reduce_sum(lsum[:, 1:2], lp[:, 1:2, 0:nkt],
                                         axis=mybir.AxisListType.X)
                    linv = spool.tile([128, 2], F32, tag="linv")
                    nc.vector.reciprocal(linv[:], lsum[:])
                    if s == 1:
                        nlam = spool.tile([128, 2], F32, tag="nlam")
                        nc.vector.tensor_scalar_mul(
                            nlam[:], linv[:], lam_t[:, 0:1])
                        linv = nlam
                    for u in range(2):
                        if s == 0:
                            ysv[u] = yspool.tile([128, OF], F32, tag="ys",
                                                 name=f"ys{u}")
                            nc.scalar.mul(ysv[u][:], op[u][:],
                                          linv[:, u:u + 1])
                        else:
                            yb = ypool.tile([128, OF], F32, tag="yb")
                            nc.scalar.mul(yb[:], op[u][:], linv[:, u:u + 1])
                            yf = ypool.tile([128, OF], F32, tag="yf")
                            nc.vector.tensor_tensor(yf[:], ysv[u][:], yb[:],
                                                    ALU.subtract)
                            t0 = qc * 256 + u * 128
                            nc.sync.dma_start(out[b, t0:t0 + 128, :], yf[:])
    nc.compile()
    return nc


def make_in_maps(x, Wq, Wk, Wv, lam, T):
    """Host-side sharding + fp8/f16 layout prep. Returns list of 8 in_maps."""
    NT = T // 128
    xf = np.ascontiguousarray(x.reshape(B * T, D).T).astype(np.float32)
    x8 = q8(xf)
    xr = q8(xf - x8.astype(np.float32))
    t = np.arange(T, dtype=np.float64)
    inv = 1.0 / (10000.0 ** (np.arange(0, HEAD_DIM, 2, dtype=np.float64)
                             / HEAD_DIM))
    fr = np.outer(t, inv)                                    # [T, 128]
    cos = np.cos(fr).astype(np.float32)
    sin = np.sin(fr).astype(np.float32)
    # [128, NT*128]: row p, col tt*128+f  ->  cos[tt*128+p, f]
    cos_sb = np.ascontiguousarray(
        cos.reshape(NT, 128, 128).transpose(1, 0, 2).reshape(128, NT * 128)
    ).astype(np.float16)
    sin_sb = np.ascontiguousarray(
        sin.reshape(NT, 128, 128).transpose(1, 0, 2).reshape(128, NT * 128)
    ).astype(np.float16)
    ident = np.eye(128, dtype=np.float16)
    ones1 = np.ones((128, 1), np.float16)
    ii = np.arange(128).reshape(128, 1)
    mm_ = np.arange(384).reshape(1, 384) - 128
    maskneg = np.where(mm_ >= ii, 0.0, NEG).astype(np.float32)
    lam_np = np.full((128, 1), lam, np.float32)

    common = {"x8d": x8, "xrd": xr, "cosd": cos_sb, "sind": sin_sb,
              "identd": ident, "onesd": ones1, "maskd": maskneg,
              "lamd": lam_np}
    in_maps = []
    for i in range(8):
        def shards(W, half):
            sh = np.concatenate(
                [W[i * 256:(i + 1) * 256], W[(i + 8) * 256:(i + 9) * 256]], 0
            ) if half else W[i * 512:(i + 1) * 512]
            wT = np.ascontiguousarray(sh.T).astype(np.float32) * WS
            w8 = q8(wT)
            wr = q8(wT - w8.astype(np.float32))
            return w8, wr
        m = dict(common)
        m["wq8"], m["wqr"] = shards(np.asarray(Wq), True)
        m["wk8"], m["wkr"] = shards(np.asarray(Wk), True)
        m["wv8"], m["wvr"] = shards(np.asarray(Wv), False)
        in_maps.append(m)
    return in_maps


_NC_CACHE: dict = {}


def run_cores(x, Wq, Wk, Wv, lambda_q1, lambda_k1, lambda_q2, lambda_k2,
              T=2048, **spmd_kwargs):
    lam1 = np.exp(np.float32(np.dot(lambda_q1.astype(np.float32),
                                    lambda_k1.astype(np.float32))))
    lam2 = np.exp(np.float32(np.dot(lambda_q2.astype(np.float32),
                                    lambda_k2.astype(np.float32))))
    lam = np.float32(lam1 - lam2 + np.float32(LAMBDA_INIT))
    if T not in _NC_CACHE:
        _NC_CACHE[T] = build_nc(T)
    nc = _NC_CACHE[T]
    in_maps = make_in_maps(np.asarray(x), np.asarray(Wq), np.asarray(Wk),
                           np.asarray(Wv), lam, T)
    res = run_bass_kernel_spmd(nc, in_maps, core_ids=list(range(8)),
                               **spmd_kwargs)
    shards = [res.results[i]["out"] for i in range(8)]       # [B,T,512] each
    y = np.stack(shards, axis=2).reshape(B, T, N_HEADS * HEAD_DIM)
    return y, res


def kernel(x, Wq, Wk, Wv, lambda_q1, lambda_k1, lambda_q2, lambda_k2):
    y, _ = run_cores(x, Wq, Wk, Wv, lambda_q1, lambda_k1, lambda_q2,
                     lambda_k2, T=x.shape[1])
    return y.astype(np.float32)
